# revision 10
# baseline (speedup 1.0000x reference)
"""MultiHeadAttention Trainium2 kernel.

Problem (hardcoded): S=2048, B=2, D=1024, H=16, HD=64, fp32 I/O.
  q = query @ w_q.T + b_q   (same for k, v), heads split from D
  scores[i,j,b,h] = (q_i . k_j)/8, masked where mask[j]==0, softmax over j
  out[i,b,:] = concat_h( sum_j p_ij v_j )

Sharding: 8 cores = 2 batches x 4 head-groups (4 heads / 256 dims each).
Host-side prep: cast to bf16, transpose to [D, seq] layout, and compact the
key/value sequence to the unmasked positions only (masked j contribute
exactly 0 after softmax), padded to a multiple of 128.

Per-core program (Tile framework). The PE write port (128 lanes) is the
hard floor for the score stream; PV is packed two heads per slot as M=64
column tiles (base partitions 0/64 -> concurrent sub-array matmuls), with
softmax denominators computed by a separate 4-up column-tiled M=1 stream
(lhsT = per-j-tile mask column) instead of a 65th V column. Outputs leave
unnormalized; the host folds the division into unsharding.

Schedule: input DMAs ordered so K/Q projections (k-outer) start ~5us in and
the first score block's exp fires as soon as ~4MB have landed. Exp phases
(ib, head-pair) pipeline: each phase's PE window carries the next phases'
projection/V passes and earlier phases' PV+denominator slots as fillers.
"""

import math
import sys

sys.path.insert(0, "/opt/trn_rl_repo")

import numpy as np
import ml_dtypes

import concourse.tile as tile
from concourse import bacc, mybir
from concourse.bass_utils import run_bass_kernel_spmd

S, B, D, H, HD = 2048, 2, 1024, 16, 64
N_CORES = 8
GROUPS = 4          # head groups (cores per batch)
GH = H // GROUPS    # heads per core = 4
GD = GH * HD        # dims per core = 256
KT = D // 128       # contraction k-tiles = 8
IBLK = 1024         # i block (exp granularity / P^T tile width)
NIB = S // IBLK     # i blocks = 2

BF16 = mybir.dt.bfloat16
F32 = mybir.dt.float32
EXP = mybir.ActivationFunctionType.Exp

_CACHE = {}


def _chunks(total, step):
    out = []
    o = 0
    while o < total:
        n = min(step, total - o)
        out.append((o, n))
        o += n
    return out


def _build(J, J_real, use_bias):
    """Build + compile the per-core Bass program (identical on all cores)."""
    NJT = J // 128
    nc = bacc.Bacc("TRN2", target_bir_lowering=False, debug=False,
                   enable_asserts=False)

    xq_d = nc.dram_tensor("xq", (D, S), BF16, kind="ExternalInput")
    xk_d = nc.dram_tensor("xk", (D, J), BF16, kind="ExternalInput")
    xv_d = nc.dram_tensor("xv", (D, J), BF16, kind="ExternalInput")
    wq_d = nc.dram_tensor("wq", (D, GD), BF16, kind="ExternalInput")
    wk_d = nc.dram_tensor("wk", (D, GD), BF16, kind="ExternalInput")
    wv_d = nc.dram_tensor("wv", (D, GD), BF16, kind="ExternalInput")
    mpad_d = nc.dram_tensor("mpad", (128, NJT), BF16, kind="ExternalInput")
    if use_bias:
        bq_d = nc.dram_tensor("bq", (GD, 1), F32, kind="ExternalInput")
        bk_d = nc.dram_tensor("bk", (GD, 1), F32, kind="ExternalInput")
        bv_d = nc.dram_tensor("bv", (1, GD), BF16, kind="ExternalInput")
        mrow_d = nc.dram_tensor("mrow", (1, J), BF16, kind="ExternalInput")
    # rows 0:256 = numerators (head h -> rows h*64..), 256:260 = denominators
    out_d = nc.dram_tensor("out", (256 + GH, S), F32, kind="ExternalOutput")

    # SBUF budget (per-partition bytes) for the P^T pool
    fixed_pp = (KT * S * 2                 # xq tiles
                + 2 * KT * J * 2           # xk, xv tiles
                + 3 * KT * GD * 2          # weights
                + 2 * S * 2 + 2 * J * 2    # qT/kT pool
                + NJT * GH * HD * 2        # ve64
                + 4 * 512 * 4              # out staging
                + 2 * 2048 * 4             # dn staging
                + 6 * 1024)                # consts, mpad, small, slack
    budget_pp = 188 * 1024 - fixed_pp
    pt_bufs = max(2 * NJT + 4, min(5 * NJT, budget_pp // (IBLK * 2)))

    scale = 1.0 / math.sqrt(HD)  # 0.125, folded into the exp

    jh1 = min(512, J)

    with tile.TileContext(nc) as tc:
        with (
            tc.tile_pool(name="xq", bufs=KT) as xq_p,
            tc.tile_pool(name="xk", bufs=KT) as xk_p,
            tc.tile_pool(name="xv", bufs=KT) as xv_p,
            tc.tile_pool(name="w", bufs=3) as w_p,
            tc.tile_pool(name="qk", bufs=2) as qk_p,
            tc.tile_pool(name="ve", bufs=NJT) as ve_p,
            tc.tile_pool(name="pt", bufs=pt_bufs) as pt_p,
            tc.tile_pool(name="small", bufs=12) as small_p,
            tc.tile_pool(name="dnp", bufs=2) as dn_p,
            tc.tile_pool(name="ost", bufs=4) as ost_p,
            tc.tile_pool(name="sps", bufs=2, space="PSUM") as sps_p,
            tc.tile_pool(name="pvs", bufs=2, space="PSUM") as pvs_p,
            tc.tile_pool(name="pps", bufs=2, space="PSUM") as pps_p,
        ):
            # ---- input DMAs, ordered for earliest first score block ----
            def load_w(w_d):
                w_sb = w_p.tile([128, KT * GD], BF16, tag="w", name="w_sb")
                nc.sync.dma_start(
                    w_sb[:].rearrange("p (k o) -> p k o", k=KT),
                    w_d.ap().rearrange("(k p) o -> p k o", p=128))
                return w_sb

            wk_sb = load_w(wk_d)
            xk_t = []
            for k in range(KT):
                t = xk_p.tile([128, J], BF16, tag="xk", name=f"xk{k}")
                nc.sync.dma_start(t[:, 0:jh1],
                                  xk_d.ap()[k * 128:(k + 1) * 128, 0:jh1])
                xk_t.append(t)
            wq_sb = load_w(wq_d)
            xq_t = []
            for k in range(KT):
                t = xq_p.tile([128, S], BF16, tag="xq", name=f"xq{k}")
                nc.sync.dma_start(t[:, 0:1024],
                                  xq_d.ap()[k * 128:(k + 1) * 128, 0:1024])
                xq_t.append(t)
            mpad_sb = small_p.tile([128, NJT], BF16, tag="mpad")
            nc.sync.dma_start(mpad_sb[:], mpad_d.ap())
            if use_bias:
                bq_c = small_p.tile([128, 2], F32, tag="biasq")
                nc.sync.dma_start(
                    bq_c[:].rearrange("p (o x) -> p o x", o=2),
                    bq_d.ap().rearrange("(o p) x -> p o x", p=128))
                bk_c = small_p.tile([128, 2], F32, tag="biask")
                nc.sync.dma_start(
                    bk_c[:].rearrange("p (o x) -> p o x", o=2),
                    bk_d.ap().rearrange("(o p) x -> p o x", p=128))
                bv_row = small_p.tile([1, GD], BF16, tag="bvrow")
                nc.sync.dma_start(bv_row[:], bv_d.ap())
                mrow_sb = small_p.tile([1, J], BF16, tag="mrow")
                nc.sync.dma_start(mrow_sb[:], mrow_d.ap())
            # j second half (needed by scores jt>=4 of phase 0)
            if J > jh1:
                for k in range(KT):
                    nc.sync.dma_start(xk_t[k][:, jh1:J],
                                      xk_d.ap()[k * 128:(k + 1) * 128, jh1:J])
            # V path (needed by PV slots from phase 1 on)
            wv_sb = load_w(wv_d)
            xv_t = []
            for k in range(KT):
                t = xv_p.tile([128, J], BF16, tag="xv", name=f"xv{k}")
                nc.sync.dma_start(t[:], xv_d.ap()[k * 128:(k + 1) * 128, :])
                xv_t.append(t)
            # i second half (needed by scores of phase 1)
            for k in range(KT):
                nc.sync.dma_start(xq_t[k][:, 1024:S],
                                  xq_d.ap()[k * 128:(k + 1) * 128, 1024:S])

            # prime the ACT exp table during the initial DMA window
            warm = small_p.tile([1, 8], F32, tag="warm")
            nc.vector.memset(warm[:], 0.0)
            warm2 = small_p.tile([1, 8], F32, tag="warm2")
            nc.scalar.activation(warm2[:], warm[:], EXP, scale=1.0)

            # ---- projections (k-outer accumulation passes) ----
            qT = {}   # per otile: [128, S] bf16  (o on partitions)
            kTt = {}  # per otile: [128, J] bf16

            def proj_pass(x_tiles, w_sb, dst, bias_col, ot, o0, n):
                ps = pps_p.tile([128, 512], F32, tag="pps",
                                name=f"pps{ot}{o0}")
                for k in range(KT):
                    lw = w_sb[:, k * GD + ot * 128:k * GD + (ot + 1) * 128]
                    nc.tensor.matmul(ps[:, 0:n], lhsT=lw,
                                     rhs=x_tiles[k][:, o0:o0 + n],
                                     start=(k == 0), stop=(k == KT - 1))
                if use_bias:
                    nc.vector.tensor_scalar(
                        dst[:, o0:o0 + n], ps[:, 0:n],
                        bias_col[:, ot:ot + 1], None, mybir.AluOpType.add)
                else:
                    nc.vector.tensor_copy(dst[:, o0:o0 + n], ps[:, 0:n])

            def proj_passes(x_tiles, w_sb, dst_map, bias_col, width, ot):
                dst = qk_p.tile([128, width], BF16,
                                tag="qt" if width == S else "kt",
                                name=f"qk{ot}")
                dst_map[ot] = dst
                return [
                    (1.7, (lambda o0=o0, n=n: proj_pass(
                        x_tiles, w_sb, dst, bias_col, ot, o0, n)))
                    for (o0, n) in _chunks(width, 512)
                ]

            # ---- V projection: ve64[jt] = [128 j, GH*HD] bf16 ----
            ve64 = [None] * NJT

            def v_wave(jt):
                ps = pps_p.tile([128, GD], F32, tag="pps", name=f"ppsv{jt}")
                for k in range(KT):
                    nc.tensor.matmul(
                        ps[:, :],
                        lhsT=xv_t[k][:, jt * 128:(jt + 1) * 128],
                        rhs=wv_sb[:, k * GD:(k + 1) * GD],
                        start=(k == 0),
                        stop=(k == KT - 1) and not use_bias)
                if use_bias:
                    # mask row as lhsT so padding j rows stay exactly 0
                    nc.tensor.matmul(ps[:, :],
                                     lhsT=mrow_sb[:, jt * 128:(jt + 1) * 128],
                                     rhs=bv_row[:, :], start=False, stop=True)
                ve = ve_p.tile([128, GD], BF16, tag="ve", name=f"ve{jt}")
                nc.vector.tensor_copy(ve[:], ps[:, :])
                ve64[jt] = ve

            # ---- PV + denominator slot for one (ib, hp, jt) ----
            def pv_slot(ib, hp, jt, pt, pvps, dps):
                """2x PV pair matmuls (M=64 col tiles) + 4-up M=1 denominator
                matmuls, all reading pt[(h, jt)]; accumulate over jt."""
                st, sp = (jt == 0), (jt == NJT - 1)
                for icl in range(2):
                    for hl in range(2):
                        h = hp * 2 + hl
                        nc.tensor.matmul(
                            pvps[icl][hl * 64:(hl + 1) * 64, :],
                            lhsT=ve64[jt][:, h * HD:(h + 1) * HD],
                            rhs=pt[(h, jt)][:, icl * 512:(icl + 1) * 512],
                            start=st, stop=sp)
                for hl in range(2):
                    for icl in range(2):
                        h = hp * 2 + hl
                        q = hl * 2 + icl
                        nc.tensor.matmul(
                            dps[q * 32:q * 32 + 1, :],
                            lhsT=mpad_sb[:, jt:jt + 1],
                            rhs=pt[(h, jt)][:, icl * 512:(icl + 1) * 512],
                            start=st, stop=sp,
                            tile_position=(0, q * 32))

            def pv_flush(ib, hp, pvps, dps):
                """Copy out the accumulated PV pair + denominators, DMA out."""
                for icl in range(2):
                    osb = ost_p.tile([128, 512], F32, tag="ost",
                                     name=f"o{ib}{hp}{icl}")
                    nc.vector.tensor_copy(osb[:], pvps[icl][:, :])
                    nc.sync.dma_start(
                        out_d.ap()[hp * 128:(hp + 1) * 128,
                                   ib * IBLK + icl * 512:
                                   ib * IBLK + (icl + 1) * 512],
                        osb[:])
                dn = dn_p.tile([1, 2048], F32, tag="dn", name=f"dn{ib}{hp}")
                for q in range(4):
                    nc.vector.tensor_copy(dn[0:1, q * 512:(q + 1) * 512],
                                          dps[q * 32:q * 32 + 1, :])
                for hl in range(2):
                    nc.sync.dma_start(
                        out_d.ap()[256 + hp * 2 + hl:256 + hp * 2 + hl + 1,
                                   ib * IBLK:(ib + 1) * IBLK],
                        dn[0:1, hl * 1024:(hl + 1) * 1024])

            def pv_block(ib, hp, pt):
                """All NJT pv slots + flush for a phase, as filler closures."""
                pvps = [pvs_p.tile([128, 512], F32, tag="pvs",
                                   name=f"pv{ib}{hp}{icl}")
                        for icl in range(2)]
                dps = pps_p.tile([128, 512], F32, tag="pps",
                                 name=f"dps{ib}{hp}")
                fl = [(0.66, (lambda jt=jt: pv_slot(ib, hp, jt, pt,
                                                    pvps, dps)))
                      for jt in range(NJT)]
                fl.append((0.1, lambda: pv_flush(ib, hp, pvps, dps)))
                return fl

            # ---- score + exp phase ----
            def emit_phase(ib, hp, fillers, start_jt=0):
                """Scores + exp for head pair hp of i-block ib. fillers is a
                list of (cost_us, closure); after each exp pair, pops fillers
                up to the window budget. Returns (pt map, leftover fillers)
                so unconsumed fillers carry into the next phase's windows."""
                i0 = ib * IBLK
                pt = {}
                budget = 0.0
                for jt in range(NJT):
                    for hl in range(2):
                        h = hp * 2 + hl
                        psx = sps_p.tile([128, IBLK], F32, tag="sps",
                                         name=f"s{ib}{hp}{jt}{hl}")
                        for (o, n) in _chunks(IBLK, 512):
                            nc.tensor.matmul(
                                psx[:, o:o + n],
                                lhsT=kTt[hp][hl * 64:(hl + 1) * 64,
                                             jt * 128:(jt + 1) * 128],
                                rhs=qT[hp][hl * 64:(hl + 1) * 64,
                                           i0 + o:i0 + o + n],
                                start=True, stop=True)
                        ptx = pt_p.tile([128, IBLK], BF16, tag="pt",
                                        name=f"pt{ib}{hp}{jt}{hl}")
                        nc.scalar.activation(ptx[:], psx[:], EXP, scale=scale)
                        pt[(h, jt)] = ptx
                    if jt >= start_jt:
                        budget += 1.2  # us of PE filler room per exp pair
                        while fillers and budget >= fillers[0][0]:
                            c, f = fillers.pop(0)
                            budget -= c
                            f()
                return pt, fillers

            # ---- emission schedule ----
            bqc = bq_c if use_bias else None
            bkc = bk_c if use_bias else None
            k0 = proj_passes(xk_t, wk_sb, kTt, bkc, J, 0)
            k0[0][1]()              # j 0:512
            q0 = proj_passes(xq_t, wq_sb, qT, bqc, S, 0)
            q0[0][1]()
            q0[1][1]()              # i 0:1024 -> first score block can start

            k1 = proj_passes(xk_t, wk_sb, kTt, bkc, J, 1)
            q1 = proj_passes(xq_t, wq_sb, qT, bqc, S, 1)
            v_fill = [(0.9, (lambda jt=jt: v_wave(jt)))
                      for jt in range(NJT)]

            nv0 = min(2, NJT)
            f00 = (k0[1:] + [q0[2], q0[3]] + [k1[0]] + v_fill[:nv0])
            pt00, rem = emit_phase(0, 0, f00, start_jt=1)
            f10 = rem + k1[1:] + q1 + v_fill[nv0:]
            pt10, rem = emit_phase(1, 0, f10)
            f01 = rem + pv_block(0, 0, pt00) + pv_block(1, 0, pt10)
            pt01, rem = emit_phase(0, 1, f01)
            f11 = rem + pv_block(0, 1, pt01)
            pt11, rem = emit_phase(1, 1, f11)
            for _, f in rem + pv_block(1, 1, pt11):
                f()

    nc.compile()
    return nc


def _prep_and_run(inputs, trace=False):
    query = np.asarray(inputs["query"], dtype=np.float32)
    key = np.asarray(inputs["key"], dtype=np.float32)
    value = np.asarray(inputs["value"], dtype=np.float32)
    mask = np.asarray(inputs["mask"]).reshape(S)
    w_q = np.asarray(inputs["w_q"], dtype=np.float32)
    b_q = np.asarray(inputs["b_q"], dtype=np.float32)
    w_k = np.asarray(inputs["w_k"], dtype=np.float32)
    b_k = np.asarray(inputs["b_k"], dtype=np.float32)
    w_v = np.asarray(inputs["w_v"], dtype=np.float32)
    b_v = np.asarray(inputs["b_v"], dtype=np.float32)

    use_bias = bool(np.any(b_q) or np.any(b_k) or np.any(b_v))

    # compact key/value over masked-out positions
    idx = np.nonzero(mask != 0)[0]
    J_real = int(len(idx))
    assert J_real > 0, "all positions masked: softmax undefined"
    J = max(512, ((J_real + 127) // 128) * 128)
    key_c = np.zeros((J, B, D), np.float32)
    key_c[:J_real] = key[idx]
    value_c = np.zeros((J, B, D), np.float32)
    value_c[:J_real] = value[idx]

    bf = ml_dtypes.bfloat16
    NJT = J // 128
    mflat = np.zeros(J, np.float32)
    mflat[:J_real] = 1  # mpad[p, t] = 1 iff t*128+p < J_real
    mpad = np.ascontiguousarray(mflat.reshape(NJT, 128).T).astype(bf)
    in_maps = []
    for core in range(N_CORES):
        b = core // GROUPS
        g = core % GROUPS
        hs = slice(g * GD, (g + 1) * GD)
        m = {
            "xq": np.ascontiguousarray(query[:, b, :].T).astype(bf),
            "xk": np.ascontiguousarray(key_c[:, b, :].T).astype(bf),
            "xv": np.ascontiguousarray(value_c[:, b, :].T).astype(bf),
            "wq": np.ascontiguousarray(w_q[hs, :].T).astype(bf),
            "wk": np.ascontiguousarray(w_k[hs, :].T).astype(bf),
            "wv": np.ascontiguousarray(w_v[hs, :].T).astype(bf),
            "mpad": mpad,
        }
        if use_bias:
            m["bq"] = np.ascontiguousarray(b_q[hs]).reshape(GD, 1)
            m["bk"] = np.ascontiguousarray(b_k[hs]).reshape(GD, 1)
            m["bv"] = np.ascontiguousarray(b_v[hs]).reshape(1, GD).astype(bf)
            m["mrow"] = mflat.reshape(1, J).astype(bf)
        in_maps.append(m)

    ck = (J, J_real, use_bias)
    if ck not in _CACHE:
        _CACHE[ck] = _build(J, J_real, use_bias)
    nc = _CACHE[ck]

    kwargs = {}
    if trace:
        kwargs = dict(trace=True, trace_cores=list(range(N_CORES)))
    res = run_bass_kernel_spmd(nc, in_maps, core_ids=list(range(N_CORES)),
                               **kwargs)

    out = np.empty((S, B, D), np.float32)
    for core in range(N_CORES):
        b = core // GROUPS
        g = core % GROUPS
        r = res.results[core]["out"]
        numer = r[0:256].reshape(GH, HD, S)
        den = r[256:256 + GH]
        out[:, b, g * GD:(g + 1) * GD] = (
            (numer / den[:, None, :]).reshape(GD, S).T)
    return out, res


def kernel(**inputs):
    out, _ = _prep_and_run(inputs, trace=False)
    return out


def run_traced(**inputs):
    _, res = _prep_and_run(inputs, trace=True)
    return res


# revision 20
# speedup vs baseline: 1.0653x; 1.0653x over previous
"""MultiHeadAttention Trainium2 kernel.

Problem (hardcoded): S=2048, B=2, D=1024, H=16, HD=64, fp32 I/O.
  q = query @ w_q.T + b_q   (same for k, v), heads split from D
  scores[i,j,b,h] = (q_i . k_j)/8, masked where mask[j]==0, softmax over j
  out[i,b,:] = concat_h( sum_j p_ij v_j )

Sharding: 8 cores = 2 batches x 4 head-groups (4 heads / 256 dims each).
Host-side prep: cast to bf16, transpose to [D, seq] layout, and compact the
key/value sequence to the unmasked positions only (masked j contribute
exactly 0 after softmax), padded to a multiple of 128.

Per-core program (Tile framework):
  - K/Q projections k-outer, output qT/kT[o, seq] bf16, two heads packed
    per 128 partitions. Input DMAs ordered so the first score block's
    inputs (~4MB) land first and exp starts ~16us in.
  - Scores transposed S^T[j, i] = kT.T @ qT, two heads packed in the
    128-row PE array via base-partition row tiling (d=64 each). The
    score stream is PSUM-write-port bound (128 lanes) and thus already
    optimal despite K=64.
  - P^T = exp(S^T / 8) on ACT in [128, 1024] chunks (bf16 out).
  - PV: out^T[vd, i] += V_ext[j,:].T @ P^T[j, i] with V_ext = per-head
    [64 v-cols | mask column] (M=65), fp32 PSUM accumulation over j; the
    softmax denominator falls out of the mask column; the host folds the
    division + transpose into unsharding.
  - Phases (ib, head-pair) pipeline; each phase's PE window carries later
    projections/V-waves and earlier phases' PV chains as cost-budgeted
    fillers; the final phase's PV chains run in lockstep behind its exps.
"""

import math
import sys

sys.path.insert(0, "/opt/trn_rl_repo")

import numpy as np
import ml_dtypes

import concourse.tile as tile
from concourse import bacc, mybir
from concourse.bass_utils import run_bass_kernel_spmd

S, B, D, H, HD = 2048, 2, 1024, 16, 64
N_CORES = 8
GROUPS = 4          # head groups (cores per batch)
GH = H // GROUPS    # heads per core = 4
GD = GH * HD        # dims per core = 256
KT = D // 128       # contraction k-tiles = 8
IBLK = 1024         # i block (exp granularity / P^T tile width)
NIB = S // IBLK     # i blocks = 2
VW1 = HD + 1        # per-head vext width (64 v cols + denominator col)

BF16 = mybir.dt.bfloat16
F32 = mybir.dt.float32
EXP = mybir.ActivationFunctionType.Exp

_CACHE = {}


def _chunks(total, step):
    out = []
    o = 0
    while o < total:
        n = min(step, total - o)
        out.append((o, n))
        o += n
    return out


def _build(J, J_real, use_bias):
    """Build + compile the per-core Bass program (identical on all cores)."""
    NJT = J // 128
    nc = bacc.Bacc("TRN2", target_bir_lowering=False, debug=False,
                   enable_asserts=False)

    xq_d = nc.dram_tensor("xq", (D, S), BF16, kind="ExternalInput")
    xk_d = nc.dram_tensor("xk", (D, J), BF16, kind="ExternalInput")
    xv_d = nc.dram_tensor("xv", (D, J), BF16, kind="ExternalInput")
    wq_d = nc.dram_tensor("wq", (D, GD), BF16, kind="ExternalInput")
    wk_d = nc.dram_tensor("wk", (D, GD), BF16, kind="ExternalInput")
    wv_d = nc.dram_tensor("wv", (D, GD), BF16, kind="ExternalInput")
    mpad_d = nc.dram_tensor("mpad", (128, NJT), BF16, kind="ExternalInput")
    if use_bias:
        bq_d = nc.dram_tensor("bq", (GD, 1), F32, kind="ExternalInput")
        bk_d = nc.dram_tensor("bk", (GD, 1), F32, kind="ExternalInput")
        bv_d = nc.dram_tensor("bv", (1, GD), BF16, kind="ExternalInput")
        mrow_d = nc.dram_tensor("mrow", (1, J), BF16, kind="ExternalInput")
    out_d = nc.dram_tensor("out", (GH * VW1, S), F32, kind="ExternalOutput")

    # SBUF budget (per-partition bytes) for the P^T pool
    fixed_pp = (KT * S * 2                 # xq tiles
                + 2 * KT * J * 2           # xk, xv tiles
                + 3 * KT * GD * 2          # weights
                + 2 * S * 2 + 2 * J * 2    # qT/kT pool
                + NJT * (GH * VW1 + 8) * 2   # vext
                + 4 * 512 * 4              # out staging
                + 6 * 1024)                # consts, mpad, small, slack
    budget_pp = 188 * 1024 - fixed_pp
    pt_bufs = max(2 * NJT + 4, min(5 * NJT, budget_pp // (IBLK * 2)))

    scale = 1.0 / math.sqrt(HD)  # 0.125, folded into the exp

    jh1 = min(512, J)

    with tile.TileContext(nc) as tc:
        with (
            tc.tile_pool(name="xq", bufs=KT) as xq_p,
            tc.tile_pool(name="xk", bufs=KT) as xk_p,
            tc.tile_pool(name="xv", bufs=KT) as xv_p,
            tc.tile_pool(name="w", bufs=3) as w_p,
            tc.tile_pool(name="qk", bufs=2) as qk_p,
            tc.tile_pool(name="ve", bufs=NJT) as ve_p,
            tc.tile_pool(name="pt", bufs=pt_bufs) as pt_p,
            tc.tile_pool(name="small", bufs=12) as small_p,
            tc.tile_pool(name="ost", bufs=4) as ost_p,
            tc.tile_pool(name="sps", bufs=2, space="PSUM") as sps_p,
            tc.tile_pool(name="pps", bufs=2, space="PSUM") as pps_p,
            tc.tile_pool(name="pvt", bufs=2, space="PSUM") as pvt_p,
        ):
            # ---- input DMAs, ordered for earliest first score block ----
            def load_w(w_d):
                w_sb = w_p.tile([128, KT * GD], BF16, tag="w", name="w_sb")
                nc.sync.dma_start(
                    w_sb[:].rearrange("p (k o) -> p k o", k=KT),
                    w_d.ap().rearrange("(k p) o -> p k o", p=128))
                return w_sb

            wk_sb = load_w(wk_d)
            xk_t = []
            for k in range(KT):
                t = xk_p.tile([128, J], BF16, tag="xk", name=f"xk{k}")
                nc.sync.dma_start(t[:, 0:jh1],
                                  xk_d.ap()[k * 128:(k + 1) * 128, 0:jh1])
                xk_t.append(t)
            wq_sb = load_w(wq_d)
            xq_t = []
            for k in range(KT):
                t = xq_p.tile([128, S], BF16, tag="xq", name=f"xq{k}")
                nc.sync.dma_start(t[:, 0:1024],
                                  xq_d.ap()[k * 128:(k + 1) * 128, 0:1024])
                xq_t.append(t)
            mpad_sb = small_p.tile([128, NJT], BF16, tag="mpad")
            nc.sync.dma_start(mpad_sb[:], mpad_d.ap())
            if use_bias:
                bq_c = small_p.tile([128, 2], F32, tag="biasq")
                nc.sync.dma_start(
                    bq_c[:].rearrange("p (o x) -> p o x", o=2),
                    bq_d.ap().rearrange("(o p) x -> p o x", p=128))
                bk_c = small_p.tile([128, 2], F32, tag="biask")
                nc.sync.dma_start(
                    bk_c[:].rearrange("p (o x) -> p o x", o=2),
                    bk_d.ap().rearrange("(o p) x -> p o x", p=128))
                bv_row = small_p.tile([1, GD], BF16, tag="bvrow")
                nc.sync.dma_start(bv_row[:], bv_d.ap())
                mrow_sb = small_p.tile([1, J], BF16, tag="mrow")
                nc.sync.dma_start(mrow_sb[:], mrow_d.ap())
            # j second half (needed by scores jt>=4 of phase 0)
            if J > jh1:
                for k in range(KT):
                    nc.sync.dma_start(xk_t[k][:, jh1:J],
                                      xk_d.ap()[k * 128:(k + 1) * 128, jh1:J])
            # V path (needed by PV chains from phase 2 on)
            wv_sb = load_w(wv_d)
            xv_t = []
            for k in range(KT):
                t = xv_p.tile([128, J], BF16, tag="xv", name=f"xv{k}")
                nc.sync.dma_start(t[:], xv_d.ap()[k * 128:(k + 1) * 128, :])
                xv_t.append(t)
            # i second half (needed by scores of phase 1)
            for k in range(KT):
                nc.sync.dma_start(xq_t[k][:, 1024:S],
                                  xq_d.ap()[k * 128:(k + 1) * 128, 1024:S])

            # prime the ACT exp table during the initial DMA window
            warm = small_p.tile([1, 8], F32, tag="warm")
            nc.vector.memset(warm[:], 0.0)
            warm2 = small_p.tile([1, 8], F32, tag="warm2")
            nc.scalar.activation(warm2[:], warm[:], EXP, scale=1.0)

            # ---- projections (k-outer accumulation passes) ----
            qT = {}   # per otile: [128, S] bf16  (o on partitions)
            kTt = {}  # per otile: [128, J] bf16

            def proj_pass(x_tiles, w_sb, dst, bias_col, ot, o0, n):
                ps = pps_p.tile([128, 512], F32, tag="pps",
                                name=f"pps{ot}{o0}")
                for k in range(KT):
                    lw = w_sb[:, k * GD + ot * 128:k * GD + (ot + 1) * 128]
                    nc.tensor.matmul(ps[:, 0:n], lhsT=lw,
                                     rhs=x_tiles[k][:, o0:o0 + n],
                                     start=(k == 0), stop=(k == KT - 1))
                if use_bias:
                    nc.vector.tensor_scalar(
                        dst[:, o0:o0 + n], ps[:, 0:n],
                        bias_col[:, ot:ot + 1], None, mybir.AluOpType.add)
                else:
                    nc.vector.tensor_copy(dst[:, o0:o0 + n], ps[:, 0:n])

            def proj_passes(x_tiles, w_sb, dst_map, bias_col, width, ot):
                dst = qk_p.tile([128, width], BF16,
                                tag="qt" if width == S else "kt",
                                name=f"qk{ot}")
                dst_map[ot] = dst
                return [
                    (1.7, (lambda o0=o0, n=n: proj_pass(
                        x_tiles, w_sb, dst, bias_col, ot, o0, n)))
                    for (o0, n) in _chunks(width, 512)
                ]

            # ---- V projection + V_ext assembly ----
            vext = [None] * NJT

            def v_wave(jt):
                ps = pps_p.tile([128, GD], F32, tag="pps", name=f"ppsv{jt}")
                for k in range(KT):
                    nc.tensor.matmul(
                        ps[:, :],
                        lhsT=xv_t[k][:, jt * 128:(jt + 1) * 128],
                        rhs=wv_sb[:, k * GD:(k + 1) * GD],
                        start=(k == 0),
                        stop=(k == KT - 1) and not use_bias)
                if use_bias:
                    # mask row as lhsT so padding j rows stay exactly 0
                    nc.tensor.matmul(ps[:, :],
                                     lhsT=mrow_sb[:, jt * 128:(jt + 1) * 128],
                                     rhs=bv_row[:, :], start=False, stop=True)
                ve = ve_p.tile([128, GH * VW1], BF16, tag="ve",
                               name=f"ve{jt}")
                for h in range(GH):
                    nc.vector.tensor_copy(
                        ve[:, h * VW1:h * VW1 + HD],
                        ps[:, h * HD:(h + 1) * HD])
                    nc.vector.tensor_copy(
                        ve[:, h * VW1 + HD:h * VW1 + HD + 1],
                        mpad_sb[:, jt:jt + 1])
                vext[jt] = ve

            # ---- PV chain for one (ib, h, icl): M=65, accumulate over j ----
            def pv_chain(ib, hp, hl, icl, pt, pool=None):
                h = hp * 2 + hl
                pv = (pool or pps_p).tile([VW1, 512], F32,
                                          tag="pps" if pool is None else "pvt",
                                          name=f"pv{ib}{h}{icl}")
                for jt in range(NJT):
                    nc.tensor.matmul(
                        pv[:, :],
                        lhsT=vext[jt][:, h * VW1:(h + 1) * VW1],
                        rhs=pt[(h, jt)][:, icl * 512:(icl + 1) * 512],
                        start=(jt == 0), stop=(jt == NJT - 1))
                _pv_out(ib, h, icl, pv)

            def _pv_out(ib, h, icl, pv):
                osb = ost_p.tile([VW1, 512], F32, tag="ost",
                                 name=f"o{ib}{h}{icl}")
                nc.vector.tensor_copy(osb[:], pv[:, :])
                nc.sync.dma_start(
                    out_d.ap()[h * VW1:(h + 1) * VW1,
                               ib * IBLK + icl * 512:
                               ib * IBLK + (icl + 1) * 512],
                    osb[:])

            def pv_block(ib, hp, pt):
                return [(1.8, (lambda hl=hl, icl=icl: pv_chain(
                    ib, hp, hl, icl, pt)))
                        for hl in range(2) for icl in range(2)]

            # ---- score + exp phase ----
            def emit_phase(ib, hp, fillers, start_jt=0, lockstep=False):
                """Scores + exp for head pair hp of i-block ib. The two
                heads' score matmuls co-issue as diagonal M=64 sub-array
                pairs. fillers: (cost_us, closure) popped per exp-pair
                window by budget; leftovers returned. lockstep: run own
                PV chains per jt (final phase)."""
                i0 = ib * IBLK
                pt = {}
                pvs = None
                if lockstep:
                    # two chains (icl=0 of both heads) lockstep behind the
                    # exps from a dedicated pool; icl=1 chains trail after
                    pvs = [pvt_p.tile([VW1, 512], F32, tag="pvt",
                                      name=f"pvt{hp}{hl}")
                           for hl in range(2)]
                budget = 0.0
                for jt in range(NJT):
                    for hl in range(2):
                        h = hp * 2 + hl
                        psx = sps_p.tile([128, IBLK], F32, tag="sps",
                                         name=f"s{ib}{hp}{jt}{hl}")
                        for (o, n) in _chunks(IBLK, 512):
                            nc.tensor.matmul(
                                psx[:, o:o + n],
                                lhsT=kTt[hp][hl * 64:(hl + 1) * 64,
                                             jt * 128:(jt + 1) * 128],
                                rhs=qT[hp][hl * 64:(hl + 1) * 64,
                                           i0 + o:i0 + o + n],
                                start=True, stop=True)
                        ptx = pt_p.tile([128, IBLK], BF16, tag="pt",
                                        name=f"pt{ib}{hp}{jt}{hl}")
                        nc.scalar.activation(ptx[:], psx[:], EXP,
                                             scale=scale)
                        pt[(h, jt)] = ptx
                    if lockstep:
                        st, sp = (jt == 0), (jt == NJT - 1)
                        for hl in range(2):
                            h = hp * 2 + hl
                            nc.tensor.matmul(
                                pvs[hl][:, :],
                                lhsT=vext[jt][:, h * VW1:(h + 1) * VW1],
                                rhs=pt[(h, jt)][:, 0:512],
                                start=st, stop=sp)
                    if jt >= start_jt:
                        budget += 1.2  # us of PE filler room per exp pair
                        while fillers and budget >= fillers[0][0]:
                            c, f = fillers.pop(0)
                            budget -= c
                            f()
                if lockstep:
                    for hl in range(2):
                        _pv_out(ib, hp * 2 + hl, 0, pvs[hl])
                    for hl in range(2):
                        pv_chain(ib, hp, hl, 1, pt, pool=pvt_p)
                return pt, fillers

            # ---- emission schedule ----
            bqc = bq_c if use_bias else None
            bkc = bk_c if use_bias else None
            k0 = proj_passes(xk_t, wk_sb, kTt, bkc, J, 0)
            k0[0][1]()              # j 0:512
            q0 = proj_passes(xq_t, wq_sb, qT, bqc, S, 0)
            q0[0][1]()
            q0[1][1]()              # i 0:1024 -> first score block can start

            k1 = proj_passes(xk_t, wk_sb, kTt, bkc, J, 1)
            q1 = proj_passes(xq_t, wq_sb, qT, bqc, S, 1)
            v_fill = [(0.9, (lambda jt=jt: v_wave(jt)))
                      for jt in range(NJT)]

            f00 = k0[1:] + [q0[2], q0[3]] + k1 + [q1[0]]
            pt00, rem = emit_phase(0, 0, f00, start_jt=1)
            f10 = rem + q1[1:] + v_fill
            pt10, rem = emit_phase(1, 0, f10)
            b10 = pv_block(1, 0, pt10)
            f01 = rem + pv_block(0, 0, pt00) + b10[:2]
            pt01, rem = emit_phase(0, 1, f01)
            f11 = rem + b10[2:] + pv_block(0, 1, pt01)
            pt11, rem = emit_phase(1, 1, f11, lockstep=True)
            for _, f in rem:
                f()

    nc.compile()
    return nc


def _prep_and_run(inputs, trace=False):
    query = np.asarray(inputs["query"], dtype=np.float32)
    key = np.asarray(inputs["key"], dtype=np.float32)
    value = np.asarray(inputs["value"], dtype=np.float32)
    mask = np.asarray(inputs["mask"]).reshape(S)
    w_q = np.asarray(inputs["w_q"], dtype=np.float32)
    b_q = np.asarray(inputs["b_q"], dtype=np.float32)
    w_k = np.asarray(inputs["w_k"], dtype=np.float32)
    b_k = np.asarray(inputs["b_k"], dtype=np.float32)
    w_v = np.asarray(inputs["w_v"], dtype=np.float32)
    b_v = np.asarray(inputs["b_v"], dtype=np.float32)

    use_bias = bool(np.any(b_q) or np.any(b_k) or np.any(b_v))

    # compact key/value over masked-out positions
    idx = np.nonzero(mask != 0)[0]
    J_real = int(len(idx))
    assert J_real > 0, "all positions masked: softmax undefined"
    J = max(512, ((J_real + 127) // 128) * 128)
    key_c = np.zeros((J, B, D), np.float32)
    key_c[:J_real] = key[idx]
    value_c = np.zeros((J, B, D), np.float32)
    value_c[:J_real] = value[idx]

    bf = ml_dtypes.bfloat16
    NJT = J // 128
    mflat = np.zeros(J, np.float32)
    mflat[:J_real] = 1  # mpad[p, t] = 1 iff t*128+p < J_real
    mpad = np.ascontiguousarray(mflat.reshape(NJT, 128).T).astype(bf)
    in_maps = []
    for core in range(N_CORES):
        b = core // GROUPS
        g = core % GROUPS
        hs = slice(g * GD, (g + 1) * GD)
        m = {
            "xq": np.ascontiguousarray(query[:, b, :].T).astype(bf),
            "xk": np.ascontiguousarray(key_c[:, b, :].T).astype(bf),
            "xv": np.ascontiguousarray(value_c[:, b, :].T).astype(bf),
            "wq": np.ascontiguousarray(w_q[hs, :].T).astype(bf),
            "wk": np.ascontiguousarray(w_k[hs, :].T).astype(bf),
            "wv": np.ascontiguousarray(w_v[hs, :].T).astype(bf),
            "mpad": mpad,
        }
        if use_bias:
            m["bq"] = np.ascontiguousarray(b_q[hs]).reshape(GD, 1)
            m["bk"] = np.ascontiguousarray(b_k[hs]).reshape(GD, 1)
            m["bv"] = np.ascontiguousarray(b_v[hs]).reshape(1, GD).astype(bf)
            m["mrow"] = mflat.reshape(1, J).astype(bf)
        in_maps.append(m)

    ck = (J, J_real, use_bias)
    if ck not in _CACHE:
        _CACHE[ck] = _build(J, J_real, use_bias)
    nc = _CACHE[ck]

    kwargs = {}
    if trace:
        kwargs = dict(trace=True, trace_cores=list(range(N_CORES)))
    res = run_bass_kernel_spmd(nc, in_maps, core_ids=list(range(N_CORES)),
                               **kwargs)

    out = np.empty((S, B, D), np.float32)
    for core in range(N_CORES):
        b = core // GROUPS
        g = core % GROUPS
        r = res.results[core]["out"].reshape(GH, VW1, S)
        out[:, b, g * GD:(g + 1) * GD] = (
            (r[:, :HD, :] / r[:, HD:HD + 1, :])     # softmax denominator
            .reshape(GD, S).T)
    return out, res


def kernel(**inputs):
    out, _ = _prep_and_run(inputs, trace=False)
    return out


def run_traced(**inputs):
    _, res = _prep_and_run(inputs, trace=True)
    return res


# revision 23
# speedup vs baseline: 1.0950x; 1.0278x over previous
"""MultiHeadAttention Trainium2 kernel.

Problem (hardcoded): S=2048, B=2, D=1024, H=16, HD=64, fp32 I/O.
  q = query @ w_q.T + b_q   (same for k, v), heads split from D
  scores[i,j,b,h] = (q_i . k_j)/8, masked where mask[j]==0, softmax over j
  out[i,b,:] = concat_h( sum_j p_ij v_j )

Sharding: 8 cores = 2 batches x 4 head-groups (4 heads / 256 dims each).
Host-side prep: cast to bf16, transpose to [D, seq] layout, and compact the
key/value sequence to the unmasked positions only (masked j contribute
exactly 0 after softmax), padded to a multiple of 128.

Per-core program (Tile framework):
  - K/Q projections k-outer, output qT/kT[o, seq] bf16, two heads packed
    per 128 partitions. Input DMAs ordered so the first score block's
    inputs (~4MB) land first and exp starts ~16us in.
  - Scores transposed S^T[j, i] = kT.T @ qT, two heads packed in the
    128-row PE array via base-partition row tiling (d=64 each). The
    score stream is PSUM-write-port bound (128 lanes) and thus already
    optimal despite K=64.
  - P^T = exp(S^T / 8) on ACT in [128, 1024] chunks (bf16 out).
  - PV: out^T[vd, i] += V_ext[j,:].T @ P^T[j, i] with V_ext = per-head
    [64 v-cols | mask column] (M=65), fp32 PSUM accumulation over j; the
    softmax denominator falls out of the mask column; the host folds the
    division + transpose into unsharding.
  - Phases (ib, head-pair) pipeline; each phase's PE window carries later
    projections/V-waves and earlier phases' PV chains as cost-budgeted
    fillers; the final phase's PV chains run in lockstep behind its exps.
"""

import math
import sys

sys.path.insert(0, "/opt/trn_rl_repo")

import numpy as np
import ml_dtypes

import concourse.tile as tile
from concourse import bacc, mybir
from concourse.bass_utils import run_bass_kernel_spmd

S, B, D, H, HD = 2048, 2, 1024, 16, 64
N_CORES = 8
GROUPS = 4          # head groups (cores per batch)
GH = H // GROUPS    # heads per core = 4
GD = GH * HD        # dims per core = 256
KT = D // 128       # contraction k-tiles = 8
IBLK = 1024         # i block (exp granularity / P^T tile width)
NIB = S // IBLK     # i blocks = 2
VW1 = HD + 1        # per-head vext width (64 v cols + denominator col)

BF16 = mybir.dt.bfloat16
F32 = mybir.dt.float32
EXP = mybir.ActivationFunctionType.Exp

_CACHE = {}


def _chunks(total, step):
    out = []
    o = 0
    while o < total:
        n = min(step, total - o)
        out.append((o, n))
        o += n
    return out


def _build(J, J_real, use_bias):
    """Build + compile the per-core Bass program (identical on all cores)."""
    NJT = J // 128
    nc = bacc.Bacc("TRN2", target_bir_lowering=False, debug=False,
                   enable_asserts=False)

    xq_d = nc.dram_tensor("xq", (D, S), BF16, kind="ExternalInput")
    xk_d = nc.dram_tensor("xk", (D, J), BF16, kind="ExternalInput")
    xv_d = nc.dram_tensor("xv", (D, J), BF16, kind="ExternalInput")
    wq_d = nc.dram_tensor("wq", (D, GD), BF16, kind="ExternalInput")
    wk_d = nc.dram_tensor("wk", (D, GD), BF16, kind="ExternalInput")
    wv_d = nc.dram_tensor("wv", (D, GD), BF16, kind="ExternalInput")
    mpad_d = nc.dram_tensor("mpad", (128, NJT), BF16, kind="ExternalInput")
    if use_bias:
        bq_d = nc.dram_tensor("bq", (GD, 1), F32, kind="ExternalInput")
        bk_d = nc.dram_tensor("bk", (GD, 1), F32, kind="ExternalInput")
        bv_d = nc.dram_tensor("bv", (1, GD), BF16, kind="ExternalInput")
        mrow_d = nc.dram_tensor("mrow", (1, J), BF16, kind="ExternalInput")
    out_d = nc.dram_tensor("out", (GH * VW1, S), F32, kind="ExternalOutput")

    # SBUF budget (per-partition bytes) for the P^T pool
    fixed_pp = (KT * S * 2                 # xq tiles
                + 2 * KT * J * 2           # xk, xv tiles
                + 3 * KT * GD * 2          # weights
                + 2 * S * 2 + 2 * J * 2    # qT/kT pool
                + NJT * (GH * VW1 + 8) * 2   # vext
                + 4 * 512 * 4              # out staging
                + 6 * 1024)                # consts, mpad, small, slack
    budget_pp = 188 * 1024 - fixed_pp
    pt_bufs = max(2 * NJT + 4, min(5 * NJT, budget_pp // (IBLK * 2)))

    scale = 1.0 / math.sqrt(HD)  # 0.125, folded into the exp

    jh1 = min(512, J)

    with tile.TileContext(nc) as tc:
        with (
            tc.tile_pool(name="xq", bufs=1) as xq_p,
            tc.tile_pool(name="xk", bufs=1) as xk_p,
            tc.tile_pool(name="xv", bufs=1) as xv_p,
            tc.tile_pool(name="w", bufs=3) as w_p,
            tc.tile_pool(name="qk", bufs=2) as qk_p,
            tc.tile_pool(name="ve", bufs=NJT) as ve_p,
            tc.tile_pool(name="pt", bufs=pt_bufs) as pt_p,
            tc.tile_pool(name="small", bufs=12) as small_p,
            tc.tile_pool(name="ost", bufs=4) as ost_p,
            tc.tile_pool(name="sps", bufs=2, space="PSUM") as sps_p,
            tc.tile_pool(name="pps", bufs=2, space="PSUM") as pps_p,
            tc.tile_pool(name="pvt", bufs=2, space="PSUM") as pvt_p,
        ):
            # ---- input DMAs, ordered for earliest first score block ----
            def load_w(w_d):
                w_sb = w_p.tile([128, KT * GD], BF16, tag="w", name="w_sb")
                nc.sync.dma_start(
                    w_sb[:].rearrange("p (k o) -> p k o", k=KT),
                    w_d.ap().rearrange("(k p) o -> p k o", p=128))
                return w_sb

            # big single-descriptor loads: per-descriptor cost dominates
            # DMA issue, so [128, KT*width] tiles load whole tensors at once
            def load_x(x_d, pool, width, tag, splits):
                x_sb = pool.tile([128, KT * width], BF16, tag=tag,
                                 name=f"{tag}_sb")
                for (o, n) in splits:
                    nc.sync.dma_start(
                        x_sb[:].rearrange("p (k w) -> p k w", k=KT)[:, :, o:o + n],
                        x_d.ap().rearrange("(k p) w -> p k w", p=128)[:, :, o:o + n])
                return [x_sb[:, k * width:(k + 1) * width] for k in range(KT)]

            wk_sb = load_w(wk_d)
            xk_t = load_x(xk_d, xk_p, J, "xk", [(0, J)])
            wq_sb = load_w(wq_d)
            xq_t = load_x(xq_d, xq_p, S, "xq", [(0, 1024), (1024, 1024)])
            mpad_sb = small_p.tile([128, NJT], BF16, tag="mpad")
            nc.sync.dma_start(mpad_sb[:], mpad_d.ap())
            if use_bias:
                bq_c = small_p.tile([128, 2], F32, tag="biasq")
                nc.sync.dma_start(
                    bq_c[:].rearrange("p (o x) -> p o x", o=2),
                    bq_d.ap().rearrange("(o p) x -> p o x", p=128))
                bk_c = small_p.tile([128, 2], F32, tag="biask")
                nc.sync.dma_start(
                    bk_c[:].rearrange("p (o x) -> p o x", o=2),
                    bk_d.ap().rearrange("(o p) x -> p o x", p=128))
                bv_row = small_p.tile([1, GD], BF16, tag="bvrow")
                nc.sync.dma_start(bv_row[:], bv_d.ap())
                mrow_sb = small_p.tile([1, J], BF16, tag="mrow")
                nc.sync.dma_start(mrow_sb[:], mrow_d.ap())
            # V path (needed by PV chains from phase 1 on)
            wv_sb = load_w(wv_d)
            xv_t = load_x(xv_d, xv_p, J, "xv", [(0, J)])

            # prime the ACT exp table during the initial DMA window
            warm = small_p.tile([1, 8], F32, tag="warm")
            nc.vector.memset(warm[:], 0.0)
            warm2 = small_p.tile([1, 8], F32, tag="warm2")
            nc.scalar.activation(warm2[:], warm[:], EXP, scale=1.0)

            # ---- projections (k-outer accumulation passes) ----
            qT = {}   # per otile: [128, S] bf16  (o on partitions)
            kTt = {}  # per otile: [128, J] bf16

            def proj_pass(x_tiles, w_sb, dst, bias_col, ot, o0, n):
                ps = pps_p.tile([128, 512], F32, tag="pps",
                                name=f"pps{ot}{o0}")
                for k in range(KT):
                    lw = w_sb[:, k * GD + ot * 128:k * GD + (ot + 1) * 128]
                    nc.tensor.matmul(ps[:, 0:n], lhsT=lw,
                                     rhs=x_tiles[k][:, o0:o0 + n],
                                     start=(k == 0), stop=(k == KT - 1))
                if use_bias:
                    nc.vector.tensor_scalar(
                        dst[:, o0:o0 + n], ps[:, 0:n],
                        bias_col[:, ot:ot + 1], None, mybir.AluOpType.add)
                else:
                    nc.vector.tensor_copy(dst[:, o0:o0 + n], ps[:, 0:n])

            def proj_passes(x_tiles, w_sb, dst_map, bias_col, width, ot):
                dst = qk_p.tile([128, width], BF16,
                                tag="qt" if width == S else "kt",
                                name=f"qk{ot}")
                dst_map[ot] = dst
                return [
                    (1.7, (lambda o0=o0, n=n: proj_pass(
                        x_tiles, w_sb, dst, bias_col, ot, o0, n)))
                    for (o0, n) in _chunks(width, 512)
                ]

            # ---- V projection + V_ext assembly ----
            vext = [None] * NJT

            def v_wave(jt):
                ps = pps_p.tile([128, GD], F32, tag="pps", name=f"ppsv{jt}")
                for k in range(KT):
                    nc.tensor.matmul(
                        ps[:, :],
                        lhsT=xv_t[k][:, jt * 128:(jt + 1) * 128],
                        rhs=wv_sb[:, k * GD:(k + 1) * GD],
                        start=(k == 0),
                        stop=(k == KT - 1) and not use_bias)
                if use_bias:
                    # mask row as lhsT so padding j rows stay exactly 0
                    nc.tensor.matmul(ps[:, :],
                                     lhsT=mrow_sb[:, jt * 128:(jt + 1) * 128],
                                     rhs=bv_row[:, :], start=False, stop=True)
                ve = ve_p.tile([128, GH * VW1], BF16, tag="ve",
                               name=f"ve{jt}")
                for h in range(GH):
                    nc.vector.tensor_copy(
                        ve[:, h * VW1:h * VW1 + HD],
                        ps[:, h * HD:(h + 1) * HD])
                    nc.vector.tensor_copy(
                        ve[:, h * VW1 + HD:h * VW1 + HD + 1],
                        mpad_sb[:, jt:jt + 1])
                vext[jt] = ve

            # ---- PV chain for one (ib, h, icl): M=65, accumulate over j ----
            def pv_chain(ib, hp, hl, icl, pt, pool=None):
                h = hp * 2 + hl
                pv = (pool or pps_p).tile([VW1, 512], F32,
                                          tag="pps" if pool is None else "pvt",
                                          name=f"pv{ib}{h}{icl}")
                for jt in range(NJT):
                    nc.tensor.matmul(
                        pv[:, :],
                        lhsT=vext[jt][:, h * VW1:(h + 1) * VW1],
                        rhs=pt[(h, jt)][:, icl * 512:(icl + 1) * 512],
                        start=(jt == 0), stop=(jt == NJT - 1))
                _pv_out(ib, h, icl, pv)

            def _pv_out(ib, h, icl, pv):
                osb = ost_p.tile([VW1, 512], F32, tag="ost",
                                 name=f"o{ib}{h}{icl}")
                nc.vector.tensor_copy(osb[:], pv[:, :])
                nc.sync.dma_start(
                    out_d.ap()[h * VW1:(h + 1) * VW1,
                               ib * IBLK + icl * 512:
                               ib * IBLK + (icl + 1) * 512],
                    osb[:])

            def pv_block(ib, hp, pt):
                return [(1.8, (lambda hl=hl, icl=icl: pv_chain(
                    ib, hp, hl, icl, pt)))
                        for hl in range(2) for icl in range(2)]

            # ---- score + exp phase ----
            def emit_phase(ib, hp, fillers, start_jt=0, lockstep=False):
                """Scores + exp for head pair hp of i-block ib. The two
                heads' score matmuls co-issue as diagonal M=64 sub-array
                pairs. fillers: (cost_us, closure) popped per exp-pair
                window by budget; leftovers returned. lockstep: run own
                PV chains per jt (final phase)."""
                i0 = ib * IBLK
                pt = {}
                pvs = None
                if lockstep:
                    # two chains (icl=0 of both heads) lockstep behind the
                    # exps from a dedicated pool; icl=1 chains trail after
                    pvs = [pvt_p.tile([VW1, 512], F32, tag="pvt",
                                      name=f"pvt{hp}{hl}")
                           for hl in range(2)]
                budget = 0.0
                for jt in range(NJT):
                    for hl in range(2):
                        h = hp * 2 + hl
                        psx = sps_p.tile([128, IBLK], F32, tag="sps",
                                         name=f"s{ib}{hp}{jt}{hl}")
                        for (o, n) in _chunks(IBLK, 512):
                            nc.tensor.matmul(
                                psx[:, o:o + n],
                                lhsT=kTt[hp][hl * 64:(hl + 1) * 64,
                                             jt * 128:(jt + 1) * 128],
                                rhs=qT[hp][hl * 64:(hl + 1) * 64,
                                           i0 + o:i0 + o + n],
                                start=True, stop=True)
                        ptx = pt_p.tile([128, IBLK], BF16, tag="pt",
                                        name=f"pt{ib}{hp}{jt}{hl}")
                        nc.scalar.activation(ptx[:], psx[:], EXP,
                                             scale=scale)
                        pt[(h, jt)] = ptx
                    if lockstep:
                        st, sp = (jt == 0), (jt == NJT - 1)
                        for hl in range(2):
                            h = hp * 2 + hl
                            nc.tensor.matmul(
                                pvs[hl][:, :],
                                lhsT=vext[jt][:, h * VW1:(h + 1) * VW1],
                                rhs=pt[(h, jt)][:, 0:512],
                                start=st, stop=sp)
                    if jt >= start_jt:
                        budget += 1.2  # us of PE filler room per exp pair
                        while fillers and budget >= fillers[0][0]:
                            c, f = fillers.pop(0)
                            budget -= c
                            f()
                if lockstep:
                    for hl in range(2):
                        _pv_out(ib, hp * 2 + hl, 0, pvs[hl])
                    for hl in range(2):
                        pv_chain(ib, hp, hl, 1, pt, pool=pvt_p)
                return pt, fillers

            # ---- emission schedule ----
            bqc = bq_c if use_bias else None
            bkc = bk_c if use_bias else None
            k0 = proj_passes(xk_t, wk_sb, kTt, bkc, J, 0)
            k0[0][1]()              # j 0:512
            q0 = proj_passes(xq_t, wq_sb, qT, bqc, S, 0)
            q0[0][1]()
            q0[1][1]()              # i 0:1024 -> first score block can start

            k1 = proj_passes(xk_t, wk_sb, kTt, bkc, J, 1)
            q1 = proj_passes(xq_t, wq_sb, qT, bqc, S, 1)
            v_fill = [(0.9, (lambda jt=jt: v_wave(jt)))
                      for jt in range(NJT)]

            for _, f in k0[1:]:
                f()                 # rest of K proj (inputs land early)
            f00 = [q0[2], q0[3]] + k1 + [q1[0], q1[1]]
            pt00, rem = emit_phase(0, 0, f00, start_jt=1)
            f10 = rem + q1[2:] + v_fill + pv_block(0, 0, pt00)
            pt10, rem = emit_phase(1, 0, f10)
            f01 = rem + pv_block(1, 0, pt10)
            pt01, rem = emit_phase(0, 1, f01)
            f11 = rem + pv_block(0, 1, pt01)
            pt11, rem = emit_phase(1, 1, f11, lockstep=True)
            for _, f in rem:
                f()

    nc.compile()
    return nc


def _prep_and_run(inputs, trace=False):
    query = np.asarray(inputs["query"], dtype=np.float32)
    key = np.asarray(inputs["key"], dtype=np.float32)
    value = np.asarray(inputs["value"], dtype=np.float32)
    mask = np.asarray(inputs["mask"]).reshape(S)
    w_q = np.asarray(inputs["w_q"], dtype=np.float32)
    b_q = np.asarray(inputs["b_q"], dtype=np.float32)
    w_k = np.asarray(inputs["w_k"], dtype=np.float32)
    b_k = np.asarray(inputs["b_k"], dtype=np.float32)
    w_v = np.asarray(inputs["w_v"], dtype=np.float32)
    b_v = np.asarray(inputs["b_v"], dtype=np.float32)

    use_bias = bool(np.any(b_q) or np.any(b_k) or np.any(b_v))

    # compact key/value over masked-out positions
    idx = np.nonzero(mask != 0)[0]
    J_real = int(len(idx))
    assert J_real > 0, "all positions masked: softmax undefined"
    J = max(512, ((J_real + 127) // 128) * 128)
    key_c = np.zeros((J, B, D), np.float32)
    key_c[:J_real] = key[idx]
    value_c = np.zeros((J, B, D), np.float32)
    value_c[:J_real] = value[idx]

    bf = ml_dtypes.bfloat16
    NJT = J // 128
    mflat = np.zeros(J, np.float32)
    mflat[:J_real] = 1  # mpad[p, t] = 1 iff t*128+p < J_real
    mpad = np.ascontiguousarray(mflat.reshape(NJT, 128).T).astype(bf)
    in_maps = []
    for core in range(N_CORES):
        b = core // GROUPS
        g = core % GROUPS
        hs = slice(g * GD, (g + 1) * GD)
        m = {
            "xq": np.ascontiguousarray(query[:, b, :].T).astype(bf),
            "xk": np.ascontiguousarray(key_c[:, b, :].T).astype(bf),
            "xv": np.ascontiguousarray(value_c[:, b, :].T).astype(bf),
            "wq": np.ascontiguousarray(w_q[hs, :].T).astype(bf),
            "wk": np.ascontiguousarray(w_k[hs, :].T).astype(bf),
            "wv": np.ascontiguousarray(w_v[hs, :].T).astype(bf),
            "mpad": mpad,
        }
        if use_bias:
            m["bq"] = np.ascontiguousarray(b_q[hs]).reshape(GD, 1)
            m["bk"] = np.ascontiguousarray(b_k[hs]).reshape(GD, 1)
            m["bv"] = np.ascontiguousarray(b_v[hs]).reshape(1, GD).astype(bf)
            m["mrow"] = mflat.reshape(1, J).astype(bf)
        in_maps.append(m)

    ck = (J, J_real, use_bias)
    if ck not in _CACHE:
        _CACHE[ck] = _build(J, J_real, use_bias)
    nc = _CACHE[ck]

    kwargs = {}
    if trace:
        kwargs = dict(trace=True, trace_cores=list(range(N_CORES)))
    res = run_bass_kernel_spmd(nc, in_maps, core_ids=list(range(N_CORES)),
                               **kwargs)

    out = np.empty((S, B, D), np.float32)
    for core in range(N_CORES):
        b = core // GROUPS
        g = core % GROUPS
        r = res.results[core]["out"].reshape(GH, VW1, S)
        out[:, b, g * GD:(g + 1) * GD] = (
            (r[:, :HD, :] / r[:, HD:HD + 1, :])     # softmax denominator
            .reshape(GD, S).T)
    return out, res


def kernel(**inputs):
    out, _ = _prep_and_run(inputs, trace=False)
    return out


def run_traced(**inputs):
    _, res = _prep_and_run(inputs, trace=True)
    return res
